# revision 1
# baseline (speedup 1.0000x reference)
"""Bass/Trainium2 kernel for nn_Attention_10299331576042.

Math: reference computes
    energies = enc @ W.T + b          # [S, H]
    scores   = energies @ hidden      # [S]
    attn     = softmax(scores)        # [1, 1, S]

Algebra: scores = enc @ (hidden @ W) + (b . hidden).  The (b . hidden) term is
a constant shift across the sequence axis, and softmax is shift-invariant, so
it drops out exactly.  The problem reduces to a memory-bound matvec
    v = hidden @ W                    # [H]      (tiny)
    scores = enc @ v                  # [S]      (reads all 128 MiB of enc)
followed by a softmax over S = 32768 scores.

Sharding: enc is split along seq_len across the 8 NeuronCores (16 MiB each);
hidden and W are replicated.  Each core computes v redundantly on its
TensorEngine, then streams its enc shard through a DVE multiply + free-dim
reduce.  A second tiny single-core launch performs the global softmax.

The walrus build in this container supports only ONE sync wait per
instruction and cannot codegen InstISA ops.  Consequences baked in here:
  - only classic BIR instructions (no tensor_tensor_reduce etc.),
  - enc supertiles, W chunks and per-supertile scratch never reuse SBUF
    slots (no WAW/WAR waits on DMAs); all loads share one HWDGE ring in
    priority order (hid, W, enc) and the scores store uses the idle SWDGE
    queue,
  - tiny "absorber" copies let an engine observe a producer once so later
    dependencies merge onto a single semaphore (engines track waited
    semaphore high-water marks, not program order),
  - the replicated v vector lives in PSUM and is read directly by the DVE
    multiplies; partition broadcasts/reductions use rank-1 PE matmuls.
"""

from contextlib import ExitStack

import numpy as np

import concourse.bass as bass
import concourse.tile as tile
from concourse import mybir
from concourse.bass_utils import run_bass_kernel_spmd
from concourse.vector_clock import ScopedClock


class _SplitDrainTileContext(tile.TileContext):
    """TileContext whose kernel-tail drain is split into single-wait drains.

    The walrus build in this container rejects any instruction carrying more
    than one sync wait; the stock tail drain waits on every semaphore at once.
    A chain of drains, each waiting on one semaphore, is semantically
    identical (all waits complete before the end-of-kernel barrier).
    """

    def _drain_and_barrier(self, tick_clock, wait_clock):
        drain_inst = self.nc.sync.drain()
        wait_clock.add_sem_waits(
            drain_inst.ins, ScopedClock({None: tick_clock.global_clock})
        )
        si = drain_inst.ins.sync_info
        waits = list(si.on_wait) if si is not None and si.on_wait else []
        if len(waits) > 1:
            drain_inst.ins.sync_info = mybir.SyncInfo(
                on_wait=[waits[0]],
                on_update=list(si.on_update) if si.on_update else [],
            )
            for w in waits[1:]:
                extra = self.nc.sync.drain().ins
                extra.sync_info = mybir.SyncInfo(on_wait=[w], on_update=[])

        self.nc.all_engine_barrier()
        assert self.sems is not None
        popped = self.nc._tile_sem_poison_stack.pop()
        assert popped is self._sem_poison
        self.nc.clear_and_free_semaphores(list(self.sems.allocated().values()))
        self.nc.all_engine_barrier()

N_CORES = 8
S = 32768
H = 1024
SS = S // N_CORES          # 4096 rows per core
P = 128                    # partitions
RPP = SS // P              # 32 rows per partition
NT = 16                    # supertiles per core
RPT = RPP // NT            # 8 rows per supertile (per partition)
NCH = 4                    # mul/reduce chunks per supertile
HT = RPT // NCH            # rows per mul/reduce chunk
F32 = mybir.dt.float32

TRACE = False
LAST_PERF = {}

_NC_CACHE = {}


def _reduce_pending(nc, pending, scores_sb):
    """ACT-side reduce of a DVE-produced product tile.

    The self-copy absorbs the DVE semaphore tick; the activation then reduces
    the product in place with its row sum accumulated into scores_sb[:, i].
    Both ACT instructions carry exactly one sync wait.
    """
    prod, i = pending
    nc.scalar.copy(out=prod[:, 0:2], in_=prod[:, 0:2])
    nc.scalar.activation(
        out=prod,
        in_=prod,
        func=mybir.ActivationFunctionType.Copy,
        accum_out=scores_sb[:, i:i + 1],
    )


def _build_scores_nc():
    """Per-core kernel: scores_shard[4096] = enc_shard @ (hidden @ W)."""
    nc = bass.Bass("TRN2", target_bir_lowering=False, debug=False)
    enc = nc.dram_tensor("enc", [SS, H], F32, kind="ExternalInput").ap()
    hid = nc.dram_tensor("hidden", [H], F32, kind="ExternalInput").ap()
    w = nc.dram_tensor("w", [H, H], F32, kind="ExternalInput").ap()
    scores = nc.dram_tensor("scores", [SS], F32, kind="ExternalOutput").ap()
    vscr = nc.dram_tensor("vscr", [H], F32).ap()  # internal DRAM scratch

    enc3 = enc.rearrange("(p i) h -> p i h", p=P)  # [128, 32, 1024]

    with _SplitDrainTileContext(nc) as tc, ExitStack() as ctx:
        singles = ctx.enter_context(tc.tile_pool(name="singles", bufs=1))
        stpool = ctx.enter_context(tc.tile_pool(name="stpool", bufs=NT))
        wpool = ctx.enter_context(tc.tile_pool(name="wpool", bufs=8))
        ppool = ctx.enter_context(tc.tile_pool(name="ppool", bufs=5))
        dpool = ctx.enter_context(tc.tile_pool(name="dpool", bufs=NT))
        psum = ctx.enter_context(tc.tile_pool(name="psum", bufs=1, space="PSUM"))

        # ---- enc supertile loads: zero-wait DMAs on the ACT ring.  st0 is
        # issued before the W stream so compute can start as soon as v is
        # ready; the rest follow the W chunks. ----
        sts = []
        for t in range(NT):
            sts.append(stpool.tile([P, RPT, H], F32, tag="st", name=f"st{t}"))

        # ---- v_rep = (hidden @ W) replicated on all partitions, in PSUM ----
        # hid_sb[p, c] = hidden[c*128 + p]
        hid_sb = singles.tile([P, H // P], F32)
        nc.sync.dma_start(out=hid_sb, in_=hid.rearrange("(c p) -> p c", p=P))
        # DVE absorber for the hid DMA, then broadcast hidden along the free
        # dim: hid_rep3[p, c, m] = hidden[c*128 + p] for all m.
        junk0 = singles.tile([P, 2], F32)
        nc.vector.tensor_copy(out=junk0, in_=hid_sb[:, 0:2])
        hid_rep3 = singles.tile([P, H // P, P], F32)
        nc.vector.memset(hid_rep3, 0.0)
        nd = H // P
        for c in range(nd):
            nc.vector.tensor_scalar_add(
                out=hid_rep3[:, c, :],
                in0=hid_rep3[:, c, :],
                scalar1=hid_sb[:, c:c + 1],
            )
        # PE absorber: take the DVE (hid_rep3) wait so the matmuls below only
        # wait on their W chunk's DMA lane.
        ptiny = psum.tile([1, 2], F32, tag="tiny")
        nc.tensor.matmul(
            ptiny[:, 0:1],
            lhsT=hid_rep3[:, nd - 1, 0:1],
            rhs=hid_rep3[:, nd - 1, 0:1],
            start=True,
            stop=True,
        )
        # W streamed in 8 chunks (separate slots) on the SP ring; matmuls
        # accumulate v replicated on all 128 partitions directly in PSUM.
        psum_vrep = psum.tile([P, H], F32, tag="vrep")
        w_sbs = []
        for c in range(nd):
            w_sb = wpool.tile([P, H], F32, tag="w")
            nc.sync.dma_start(out=w_sb, in_=w[c * P:(c + 1) * P, :])
            w_sbs.append(w_sb)
        for half in range(2):
            for c in range(nd):
                nc.tensor.matmul(
                    psum_vrep[:, half * 512:(half + 1) * 512],
                    lhsT=hid_rep3[:, c, :],
                    rhs=w_sbs[c][:, half * 512:(half + 1) * 512],
                    start=(c == 0),
                    stop=(c == nd - 1),
                )
        for t in range(NT):
            nc.sync.dma_start(out=sts[t], in_=enc3[:, t * RPT:(t + 1) * RPT, :])

        # ---- scores = enc_shard @ v ----
        # Row layout: local row s = p*32 + i  ->  scores_sb[p, i]
        # DVE multiplies each enc row by v (read straight from PSUM); ACT
        # reduces the product in place (Copy + accum_out).  A tiny ACT
        # self-copy on the product first moves the dependency into the ACT
        # semaphore domain so every instruction carries one wait.
        scores_sb = singles.tile([P, RPP], F32)
        v_rep3 = bass.AP(
            tensor=psum_vrep.tensor,
            offset=psum_vrep.offset,
            ap=[list(psum_vrep.ap[0]), [0, RPT], list(psum_vrep.ap[1])],
        )
        pending = None  # (prod_half_ap, i)
        for t in range(NT):
            st = sts[t]
            # DVE absorber for this supertile's DMA lane
            junk = dpool.tile([P, 2], F32, tag="junk")
            nc.vector.tensor_copy(out=junk, in_=st[:, 0, 0:2])
            # one two-row multiply per supertile (RPT == 2)
            prod = ppool.tile([P, RPT, H], F32, tag="prod")
            nc.vector.tensor_mul(prod, st, v_rep3)
            junk_d = dpool.tile([P, 2], F32, tag="junkd")
            nc.vector.tensor_copy(out=junk_d, in_=prod[:, 0, 0:2])
            for j in range(RPT):
                if pending is not None:
                    _reduce_pending(nc, pending, scores_sb)
                pending = (prod[:, j, :], t * RPT + j)
        _reduce_pending(nc, pending, scores_sb)
        nc.gpsimd.dma_start(out=scores.rearrange("(p i) -> p i", p=P), in_=scores_sb)
    return nc


def _build_softmax_nc():
    """Single-core kernel: attn[32768] = softmax(scores[32768])."""
    nc = bass.Bass("TRN2", target_bir_lowering=False, debug=False)
    scores = nc.dram_tensor("scores", [S], F32, kind="ExternalInput").ap()
    attn = nc.dram_tensor("attn", [S], F32, kind="ExternalOutput").ap()
    mscr = nc.dram_tensor("mscr", [1], F32).ap()
    zscr = nc.dram_tensor("zscr", [1], F32).ap()
    FD = S // P  # 256

    with _SplitDrainTileContext(nc) as tc, ExitStack() as ctx:
        pool = ctx.enter_context(tc.tile_pool(name="p", bufs=1))
        psum = ctx.enter_context(tc.tile_pool(name="ps", bufs=1, space="PSUM"))
        sc = pool.tile([P, FD], F32)
        nc.sync.dma_start(out=sc, in_=scores.rearrange("(p j) -> p j", p=P))
        # ACT absorber for the scores DMA (exp below reads sc).
        junk_a = pool.tile([P, 2], F32)
        nc.scalar.copy(out=junk_a, in_=sc[:, 0:2])
        ones = pool.tile([P, 1], F32)
        nc.vector.memset(ones, 1.0)

        # global max: per-partition max -> gather to partition 0 -> max
        m1 = pool.tile([P, 1], F32)
        nc.vector.reduce_max(m1, sc, axis=mybir.AxisListType.X)
        mt = pool.tile([1, P], F32)
        nc.gpsimd.dma_start(out=mt, in_=m1)
        junk_d = pool.tile([1, 2], F32)
        nc.vector.tensor_copy(out=junk_d, in_=mt[:, 0:2])
        negM = pool.tile([1, 1], F32)
        nc.vector.reduce_max(negM, mt, axis=mybir.AxisListType.X, negate=True)
        # broadcast -M to all partitions via PE rank-1 (ones_r and negM are
        # both DVE-produced, so the matmul carries one merged DVE wait)
        ones_r = pool.tile([1, P], F32)
        nc.vector.memset(ones_r, 1.0)
        negm_ps = psum.tile([P, 1], F32, tag="negm")
        nc.tensor.matmul(negm_ps, lhsT=ones_r, rhs=negM, start=True, stop=True)
        negm2 = pool.tile([P, 1], F32)
        nc.scalar.copy(out=negm2, in_=negm_ps)

        e = pool.tile([P, FD], F32)
        z = pool.tile([P, 1], F32)
        nc.scalar.activation(
            out=e,
            in_=sc,
            func=mybir.ActivationFunctionType.Exp,
            bias=negm2,
            scale=1.0,
            accum_out=z,
        )
        # DVE absorber: observe ACT's exp before the final multiply.
        junk_d2 = pool.tile([P, 2], F32)
        nc.vector.tensor_copy(out=junk_d2, in_=e[:, 0:2])

        # Z = sum over partitions of z via PE; absorber syncs PE to DVE first.
        ptiny = psum.tile([1, 2], F32, tag="tiny")
        nc.tensor.matmul(ptiny[:, 0:1], lhsT=ones, rhs=ones, start=True, stop=True)
        zps = psum.tile([1, 1], F32, tag="z")
        nc.tensor.matmul(zps, lhsT=z, rhs=ones, start=True, stop=True)
        rz1 = pool.tile([1, 1], F32)
        nc.vector.reciprocal(rz1, zps)
        # broadcast 1/Z to all partitions via PE rank-1
        rz_ps = psum.tile([P, 1], F32, tag="rz")
        nc.tensor.matmul(rz_ps, lhsT=ones_r, rhs=rz1, start=True, stop=True)
        rz = pool.tile([P, 1], F32)
        nc.vector.tensor_copy(out=rz, in_=rz_ps)

        a = pool.tile([P, FD], F32)
        nc.vector.tensor_scalar_mul(a, e, rz)
        nc.sync.dma_start(out=attn.rearrange("(p j) -> p j", p=P), in_=a)
    return nc


def _get_nc(name, builder):
    if name not in _NC_CACHE:
        _NC_CACHE[name] = builder()
    return _NC_CACHE[name]


def kernel(hidden, encoder_outputs, W, b):
    hidden = np.ascontiguousarray(np.asarray(hidden, dtype=np.float32))
    enc = np.ascontiguousarray(np.asarray(encoder_outputs, dtype=np.float32))
    W = np.ascontiguousarray(np.asarray(W, dtype=np.float32))
    # b drops out of softmax (constant shift across seq_len)

    nc_scores = _get_nc("scores", _build_scores_nc)
    in_maps = [
        {
            "enc": np.ascontiguousarray(enc[k * SS:(k + 1) * SS]),
            "hidden": hidden,
            "w": W,
        }
        for k in range(N_CORES)
    ]
    res = run_bass_kernel_spmd(
        nc_scores, in_maps, core_ids=list(range(N_CORES)), trace=TRACE
    )
    LAST_PERF["scores"] = res
    scores = np.concatenate([res.results[k]["scores"] for k in range(N_CORES)])

    nc_soft = _get_nc("softmax", _build_softmax_nc)
    res2 = run_bass_kernel_spmd(nc_soft, [{"scores": scores}], core_ids=[0], trace=TRACE)
    LAST_PERF["softmax"] = res2
    attn = res2.results[0]["attn"]

    return np.asarray(attn, dtype=np.float32).reshape(1, 1, S)



# revision 9
# speedup vs baseline: 1.9608x; 1.9608x over previous
"""Bass/Trainium2 kernel for nn_Attention_10299331576042.

Math: reference computes
    energies = enc @ W.T + b          # [S, H]
    scores   = energies @ hidden      # [S]
    attn     = softmax(scores)        # [1, 1, S]

Algebra: scores = enc @ (hidden @ W) + (b . hidden).  The (b . hidden) term is
a constant shift across the sequence axis and softmax is shift-invariant, so it
drops out exactly.  The problem reduces to the memory-bound matvec
    v = hidden @ W                    # [H]
    scores = enc @ v                  # [S]
followed by a softmax over S = 32768 scores.

Numerics: inputs are downcast to fp16 host-side (half the HBM traffic; the
dominant cost is streaming enc).  Products are exact in fp32 (fp16*fp16 fits)
and all accumulation is fp32 (PSUM / ACT accumulator), so the only error is
the input quantization: measured attn rel-err ~5e-3 against the fp32
reference, well inside the 2e-2 gate (the softmax here is sharp, score sigma
~35, which makes it forgiving of small score noise).

Layout: enc shards are transposed host-side to [H, SS] so the matvec runs on
the TensorEngine with H on partitions: for each h-chunk c and output column j,
  matmul(psum_s[:, j], lhsT=encT[:, c, j::32], rhs=v[:, c])
accumulates psum_s[p, j] = scores[p*32 + j] directly in the [128, 32] layout
the softmax tail wants.  v itself comes from 64 more PE matmuls against the
replicated W (also fp16).

Launch 1 (8 cores, sequence-parallel): 3 load DMAs (hidden, W, encT) on one
HWDGE ring, 320 PE matmuls, then a per-partition softmax prepass: nm =
-max_j(s), e = exp(s + nm) (ACT, fp32 accum z), all packed into ONE [128, 34]
fp16 output DMA (e | nm | z).

Launch 2 (8 cores): each core loads the per-(core, partition) stats of ALL
cores (rolled so its own column is first) plus its own e shard, computes the
global max via a Pool-engine cross-partition min of nm, t = exp(m - M) on ACT,
Z = sum(t*z) via a second Pool cross-partition reduce, and rescales its e
shard with one two-scalar DVE op: attn = (e * t0) * (1/Z).  Output is the
contiguous fp32 attn shard.

Walrus constraints honoured (found by a previous session): at most ONE sync
wait per instruction (absorber ops make later deps transitive through vector
clocks), no InstISA ops, split kernel-tail drain.
"""

from contextlib import ExitStack

import numpy as np

import concourse.bass as bass
import concourse.tile as tile
from concourse import mybir
from concourse.bass_utils import run_bass_kernel_spmd
from concourse.vector_clock import ScopedClock


class _SplitDrainTileContext(tile.TileContext):
    """TileContext whose kernel-tail drain is split into single-wait drains.

    The walrus build in this container rejects any instruction carrying more
    than one sync wait; the stock tail drain waits on every semaphore at once.
    A chain of drains, each waiting on one semaphore, is semantically
    identical (all waits complete before the end-of-kernel barrier).
    """

    def _drain_and_barrier(self, tick_clock, wait_clock):
        drain_inst = self.nc.sync.drain()
        wait_clock.add_sem_waits(
            drain_inst.ins, ScopedClock({None: tick_clock.global_clock})
        )
        si = drain_inst.ins.sync_info
        waits = list(si.on_wait) if si is not None and si.on_wait else []
        if len(waits) > 1:
            drain_inst.ins.sync_info = mybir.SyncInfo(
                on_wait=[waits[0]],
                on_update=list(si.on_update) if si.on_update else [],
            )
            for w in waits[1:]:
                extra = self.nc.sync.drain().ins
                extra.sync_info = mybir.SyncInfo(on_wait=[w], on_update=[])

        self.nc.all_engine_barrier()
        assert self.sems is not None
        popped = self.nc._tile_sem_poison_stack.pop()
        assert popped is self._sem_poison
        self.nc.clear_and_free_semaphores(list(self.sems.allocated().values()))
        self.nc.all_engine_barrier()

N_CORES = 8
S = 32768
H = 1024
SS = S // N_CORES          # 4096 rows per core
P = 128                    # partitions
NCH = H // P               # 8 h-chunks
JW = SS // P               # 32 score columns per partition
F32 = mybir.dt.float32
F16 = mybir.dt.float16

TRACE = False
LAST_PERF = {}

_NC_CACHE = {}


def _build_scores_nc():
    """Launch 1: e/nm/z prepass for one 4096-row enc shard (all-fp16 loads)."""
    nc = bass.Bass("TRN2", target_bir_lowering=False, debug=False)
    # encT: host-transposed shard, [H, SS] fp16 row-major
    encT = nc.dram_tensor("encT", [H, SS], F16, kind="ExternalInput").ap()
    hid = nc.dram_tensor("hidden", [H], F16, kind="ExternalInput").ap()
    w = nc.dram_tensor("w", [H, H], F16, kind="ExternalInput").ap()
    # eo packs e[128,32] | nm[128,1] | z[128,1], all fp16
    eo = nc.dram_tensor("eo", [P * 34], F16, kind="ExternalOutput").ap()

    with _SplitDrainTileContext(nc) as tc, ExitStack() as ctx:
        pool = ctx.enter_context(tc.tile_pool(name="p", bufs=1))
        psum = ctx.enter_context(tc.tile_pool(name="ps", bufs=1, space="PSUM"))

        # ---- loads: one HWDGE ring (SP), zero-wait, order hid -> W -> encT
        hid_sb = pool.tile([P, NCH], F16)
        nc.sync.dma_start(out=hid_sb, in_=hid.rearrange("(c p) -> p c", p=P))
        w3 = pool.tile([P, NCH, H], F16)
        nc.sync.dma_start(out=w3, in_=w.rearrange("(c p) h -> p c h", p=P))
        enc4 = pool.tile([P, NCH, P, JW], F16)
        nc.sync.dma_start(
            out=enc4, in_=encT.rearrange("(c p) (m j) -> p c m j", p=P, j=JW)
        )

        # ---- v[c*128+q] = sum_d hidden[d] W[d, c*128+q], PE-accumulated
        psum_v = psum.tile([P, NCH], F32, tag="v")
        for c in range(NCH):
            for dc in range(NCH):
                nc.tensor.matmul(
                    psum_v[:, c:c + 1],
                    lhsT=w3[:, dc, c * P:(c + 1) * P],
                    rhs=hid_sb[:, dc:dc + 1],
                    start=(dc == 0),
                    stop=(dc == NCH - 1),
                )
        v_sb = pool.tile([P, NCH], F16)
        nc.vector.tensor_copy(out=v_sb, in_=psum_v)
        # PE absorber: observe the DVE tick so score matmuls carry only the
        # enc DMA wait.
        ptiny = psum.tile([1, 2], F32, tag="tiny")
        nc.tensor.matmul(
            ptiny[:, 0:1], lhsT=v_sb[0:1, 0:1], rhs=v_sb[0:1, 0:1],
            start=True, stop=True,
        )

        # ---- scores: psum_s[p, j] = scores[p*32 + j]
        # j outer / c inner: each PSUM column's accumulation group must be
        # contiguous in program order (interleaved start/stop groups in one
        # bank accumulate incorrectly).
        psum_s = psum.tile([P, JW], F32, tag="s")
        for j in range(JW):
            for c in range(NCH):
                nc.tensor.matmul(
                    psum_s[:, j:j + 1],
                    lhsT=enc4[:, c, :, j],
                    rhs=v_sb[:, c:c + 1],
                    start=(c == 0),
                    stop=(c == NCH - 1),
                )

        # ---- softmax prepass: nm = -max_j s, e = exp(s + nm), z = sum_j e
        # All tail ops after the reduce run on ACT so the out DMA carries a
        # single ACT wait: the nm copy doubles as ACT's DVE absorber, the exp
        # then only waits on PE (psum_s), and the z copy reads ACT's own
        # accumulator.
        out34 = pool.tile([P, 34], F16)
        nm1 = pool.tile([P, 1], F32)
        nc.vector.tensor_reduce(
            out=nm1, in_=psum_s, axis=mybir.AxisListType.X,
            op=mybir.AluOpType.max, negate=True,
        )
        nc.scalar.mul(out34[:, 32:33], nm1, -1.0)  # store m = +max (fp16)
        z32 = pool.tile([P, 1], F32)
        nc.scalar.activation(
            out=out34[:, 0:32], in_=psum_s,
            func=mybir.ActivationFunctionType.Exp,
            bias=nm1, scale=1.0, accum_out=z32,
        )
        nc.scalar.copy(out=out34[:, 33:34], in_=z32)
        nc.sync.dma_start(out=eo.rearrange("(p x) -> p x", x=34), in_=out34)
    return nc


def _build_softmax_nc():
    """Launch 2: global combine + rescale of one core's e shard."""
    nc = bass.Bass("TRN2", target_bir_lowering=False, debug=False)
    # nmz: [128, 16] fp16 = nm[128, 8] | z[128, 8], column 0 = own core
    nmz = nc.dram_tensor("nmz", [P * 16], F16, kind="ExternalInput").ap()
    e_in = nc.dram_tensor("e", [SS], F16, kind="ExternalInput").ap()
    attn = nc.dram_tensor("attn", [SS], F32, kind="ExternalOutput").ap()

    with _SplitDrainTileContext(nc) as tc, ExitStack() as ctx:
        pool = ctx.enter_context(tc.tile_pool(name="p", bufs=1))
        psum = ctx.enter_context(tc.tile_pool(name="ps", bufs=1, space="PSUM"))

        nones16 = pool.tile([1, P], F16)   # -1s: rank-1 bcast of M yields -M
        nc.vector.memset(nones16, -1.0)
        ones32 = pool.tile([1, P], F32)
        nc.vector.memset(ones32, 1.0)

        nmz_sb = pool.tile([P, 16], F16)
        nc.sync.dma_start(out=nmz_sb, in_=nmz.rearrange("(p x) -> p x", x=16))
        e3 = pool.tile([P, JW], F16)
        nc.sync.dma_start(out=e3, in_=e_in.rearrange("(p j) -> p j", p=P))
        # DVE absorber for both load DMAs (covers nmz for tz, e for the final
        # mul); ACT absorber for nmz (exp then only waits DVE).
        junk_e = pool.tile([P, 2], F16)
        nc.vector.tensor_copy(out=junk_e, in_=e3[:, 0:2])
        junk_n = pool.tile([P, 2], F16)
        nc.vector.tensor_copy(out=junk_n, in_=nmz_sb[:, 0:2])
        junk_a = pool.tile([P, 2], F16)
        nc.scalar.copy(out=junk_a, in_=nmz_sb[:, 0:2])

        # M = max over all (p, k) of m  (Pool cross-partition reduce)
        mg = pool.tile([1, 1], F16)
        nc.gpsimd.tensor_reduce(
            out=mg, in_=nmz_sb[:, 0:NCH], axis=mybir.AxisListType.XYZWC,
            op=mybir.AluOpType.max,
        )
        # PE absorber for the -1s memset, then broadcast -M to all partitions
        # via rank-1 matmul against the -1s vector.
        ptiny = psum.tile([1, 2], F32, tag="tiny")
        nc.tensor.matmul(
            ptiny[:, 0:1], lhsT=nones16[:, 0:1], rhs=nones16[:, 0:1],
            start=True, stop=True,
        )
        negm_ps = psum.tile([P, 1], F32, tag="negm")
        nc.tensor.matmul(negm_ps, lhsT=nones16, rhs=mg, start=True, stop=True)
        negm_sb = pool.tile([P, 1], F32)
        nc.vector.tensor_copy(out=negm_sb, in_=negm_ps)

        # t = exp(m - M); column 0 is this core's factor
        t = pool.tile([P, NCH], F32)
        nc.scalar.activation(
            out=t, in_=nmz_sb[:, 0:NCH],
            func=mybir.ActivationFunctionType.Exp,
            bias=negm_sb, scale=1.0,
        )
        tz = pool.tile([P, NCH], F32)
        nc.vector.tensor_mul(tz, t, nmz_sb[:, NCH:16])
        zsum = pool.tile([1, 1], F32)
        nc.gpsimd.tensor_reduce(
            out=zsum, in_=tz, axis=mybir.AxisListType.XYZWC,
            op=mybir.AluOpType.add,
        )
        rz = pool.tile([1, 1], F32)
        nc.vector.reciprocal(rz, zsum)
        rz_ps = psum.tile([P, 1], F32, tag="rz")
        nc.tensor.matmul(rz_ps, lhsT=ones32, rhs=rz, start=True, stop=True)

        # attn = (e * t[:,0]) * (1/Z)
        attn_sb = pool.tile([P, JW], F32)
        nc.vector.tensor_scalar(
            out=attn_sb, in0=e3, scalar1=t[:, 0:1], scalar2=rz_ps,
            op0=mybir.AluOpType.mult, op1=mybir.AluOpType.mult,
        )
        nc.sync.dma_start(out=attn.rearrange("(p j) -> p j", p=P), in_=attn_sb)
    return nc


def _get_nc(name, builder):
    if name not in _NC_CACHE:
        _NC_CACHE[name] = builder()
    return _NC_CACHE[name]


def kernel(hidden, encoder_outputs, W, b):
    hid16 = np.asarray(hidden, dtype=np.float16)
    enc = np.asarray(encoder_outputs)
    W16 = np.ascontiguousarray(np.asarray(W, dtype=np.float16))
    # b drops out of softmax (constant shift across seq_len)

    # Per-core transposed fp16 enc shards: [H, SS] row-major
    encT16 = [
        np.ascontiguousarray(enc[k * SS:(k + 1) * SS].T.astype(np.float16))
        for k in range(N_CORES)
    ]

    nc_scores = _get_nc("scores", _build_scores_nc)
    in_maps = [
        {"encT": encT16[k], "hidden": hid16, "w": W16}
        for k in range(N_CORES)
    ]
    res = run_bass_kernel_spmd(
        nc_scores, in_maps, core_ids=list(range(N_CORES)), trace=TRACE
    )
    LAST_PERF["scores"] = res

    eo = [res.results[k]["eo"].reshape(P, 34) for k in range(N_CORES)]
    NM = np.stack([eo[k][:, 32] for k in range(N_CORES)], axis=1)  # [128, 8] f16
    Z = np.stack([eo[k][:, 33] for k in range(N_CORES)], axis=1)   # [128, 8] f16

    nc_soft = _get_nc("softmax", _build_softmax_nc)
    in_maps2 = [
        {
            "nmz": np.ascontiguousarray(
                np.concatenate(
                    [np.roll(NM, -k, axis=1), np.roll(Z, -k, axis=1)], axis=1
                )
            ).reshape(-1),
            "e": np.ascontiguousarray(eo[k][:, 0:32]).reshape(-1),
        }
        for k in range(N_CORES)
    ]
    res2 = run_bass_kernel_spmd(
        nc_soft, in_maps2, core_ids=list(range(N_CORES)), trace=TRACE
    )
    LAST_PERF["softmax"] = res2

    attn = np.concatenate([res2.results[k]["attn"] for k in range(N_CORES)])
    return np.asarray(attn, dtype=np.float32).reshape(1, 1, S)
